# revision 10
# baseline (speedup 1.0000x reference)
"""Segment-prefix max kernel for Trainium2 (8 NeuronCores, SPMD).

Problem: x [1048576, 128] f32, 2048 uniform segments of 512 rows each;
out[i, :] = max over the first (512 - window_size + 1) rows of segment i.

Strategy (memory-bound, ~512 MiB streamed from HBM):
  - Shard segments across 8 cores: core c gets rows [c*131072, (c+1)*131072)
    and produces out rows [c*256, (c+1)*256). No cross-core communication.
  - Per core, 2 MiB tiles of 8 segments. Partition 16*l + i holds rows
    [32*i, 32*i+32) of segment l — one 16 KiB contiguous DMA run per
    partition (vs 2 KiB in the naive layout). Loads round-robin over four
    HWDGE rings (sync/scalar/gpsimd/tensor) to keep all 16 DMA engines fed.
  - Run 15 of each segment includes the invalid window tail (rows 510/511),
    so it is EXCLUDED from the per-tile reduce. The valid tail rows
    [count-32, count) of all segments are instead loaded once up front
    (4 MiB), folded separately, and joined into the output columns.
  - The 32 -> 1 fold along the free axis runs on DVE as a binary tree; the
    first level reads f32 and writes bf16, middle levels run in bf16 at 2x
    DVE throughput, the last level emits f32 (rel tolerance 2e-2 >> bf16's
    ~4e-3 rounding).
  - Cross-partition max (each segment = 16 consecutive partitions) goes
    through a PE transpose (identity matmul into PSUM) and one DVE
    reduce_max over each segment's first 15 columns, max'd with the
    precomputed tail column.
  - Columns accumulate in an SBUF [128, n_seg] f32 buffer, PE-transposed
    back to row-major [n_seg, 128] chunks and DMA'd out.
"""

import numpy as np

import concourse.bacc as bacc
import concourse.bass as bass
import concourse.tile as tile
from concourse import mybir
from concourse.bass_utils import run_bass_kernel_spmd
from concourse.masks import make_identity

N_CORES = 8
SEG_LEN = 512
D = 128
J = 32  # rows per partition per tile (16 KiB contiguous DMA run)
RUNS = SEG_LEN // J  # 16 partitions per segment
SEGS_PER_TILE = 128 // RUNS  # 8 segments * 4096 rows * 512 B = 2 MiB tiles

_PROGRAM_CACHE: dict = {}


def _build_program(n_seg_core: int, count: int) -> bacc.Bacc:
    """Bass program for one core: n_seg_core segments, max over first
    `count` rows of each. Requires SEG_LEN - J < count <= SEG_LEN."""
    assert SEG_LEN - J < count <= SEG_LEN
    rows = n_seg_core * SEG_LEN
    n_tiles = n_seg_core // SEGS_PER_TILE
    n_slot = n_seg_core // 128  # tail slots per partition
    has_tail = count < SEG_LEN
    f32 = mybir.dt.float32
    bf16 = mybir.dt.bfloat16

    nc = bacc.Bacc("TRN2", target_bir_lowering=False, debug=False)
    x_in = nc.dram_tensor("x", [rows, D], f32, kind="ExternalInput")
    out_t = nc.dram_tensor("out", [n_seg_core, D], f32, kind="ExternalOutput")

    # partition-major view: partition p of tile t holds rows of run p
    x_flat = x_in.rearrange("(p j) d -> p j d", j=J)
    # tail view: partition p, slot a -> last valid rows of segment a*128+p
    x_tail = x_in.rearrange("(a p q) d -> p a q d", p=128, q=SEG_LEN)

    rings = [nc.sync, nc.scalar, nc.gpsimd]

    with tile.TileContext(nc) as tc:
        with (
            tc.tile_pool(name="io", bufs=8) as io_pool,
            tc.tile_pool(name="work", bufs=4) as work_pool,
            tc.tile_pool(name="psum", bufs=4, space="PSUM") as psum_pool,
            tc.tile_pool(name="psum2", bufs=2, space="PSUM") as psum_pool2,
            tc.tile_pool(name="consts", bufs=1) as consts,
        ):
            ident_f = consts.tile([128, 128], f32)
            make_identity(nc, ident_f)
            outbuf = consts.tile([128, n_seg_core], f32)

            if has_tail:
                # upfront: fold the valid tail rows of every segment
                tt = consts.tile([128, n_slot, J, D], f32)
                nc.gpsimd.dma_start(
                    out=tt, in_=x_tail[:, :, count - J : count]
                )
                wt = consts.tile([128, n_slot, J // 2, D], bf16)
                nc.vector.tensor_max(
                    out=wt, in0=tt[:, :, : J // 2], in1=tt[:, :, J // 2 :]
                )
                k = J // 2
                while k > 2:
                    k //= 2
                    nc.vector.tensor_max(
                        out=wt[:, :, :k], in0=wt[:, :, :k], in1=wt[:, :, k : 2 * k]
                    )
                wtf = consts.tile([128, n_slot, D], f32)
                nc.vector.tensor_max(out=wtf, in0=wt[:, :, 0], in1=wt[:, :, 1])
                taibuf = consts.tile([128, n_seg_core], f32)
                for a in range(n_slot):
                    ptt = psum_pool2.tile([128, 128], f32, tag="ptt")
                    nc.tensor.transpose(ptt, wtf[:, a], ident_f)
                    nc.scalar.copy(taibuf[:, a * 128 : (a + 1) * 128], ptt)

            for t in range(n_tiles):
                tl = io_pool.tile([128, J, D], f32, tag="tl")
                g0 = t * SEGS_PER_TILE
                rings[t % 3].dma_start(out=tl, in_=x_flat[t * 128 : (t + 1) * 128])

                # fold 32 -> 1 along j: f32 -> bf16, bf16 tree, bf16 -> f32
                w = work_pool.tile([128, J // 2, D], bf16, tag="w")
                nc.vector.tensor_max(out=w, in0=tl[:, : J // 2], in1=tl[:, J // 2 :])
                k = J // 2
                while k > 2:
                    k //= 2
                    nc.vector.tensor_max(
                        out=w[:, :k], in0=w[:, :k], in1=w[:, k : 2 * k]
                    )
                wf = work_pool.tile([128, D], f32, tag="wf")
                nc.vector.tensor_max(out=wf, in0=w[:, 0], in1=w[:, 1])

                # cross-partition max: transpose, reduce each segment's
                # valid columns, then join the tail column
                pt = psum_pool.tile([128, SEGS_PER_TILE, RUNS], f32, tag="pt")
                nc.tensor.transpose(pt.rearrange("p a b -> p (a b)"), wf, ident_f)
                if has_tail:
                    tmp = work_pool.tile([128, SEGS_PER_TILE], f32, tag="tmp")
                    nc.vector.reduce_max(
                        out=tmp, in_=pt[:, :, : RUNS - 1], axis=mybir.AxisListType.X
                    )
                    nc.vector.tensor_max(
                        out=outbuf[:, g0 : g0 + SEGS_PER_TILE],
                        in0=tmp,
                        in1=taibuf[:, g0 : g0 + SEGS_PER_TILE],
                    )
                else:
                    nc.vector.reduce_max(
                        out=outbuf[:, g0 : g0 + SEGS_PER_TILE],
                        in_=pt,
                        axis=mybir.AxisListType.X,
                    )

            # outbuf is [128 d, n_seg_core]; transpose back to [seg, d]
            for c in range(n_seg_core // 128):
                pt = psum_pool2.tile([128, 128], f32, tag="ot_ps")
                nc.tensor.transpose(pt, outbuf[:, c * 128 : (c + 1) * 128], ident_f)
                ot = io_pool.tile([128, 128], f32, tag="ot")
                nc.scalar.copy(ot, pt)
                nc.sync.dma_start(out=out_t[c * 128 : (c + 1) * 128, :], in_=ot)
    nc.compile()
    return nc


def _numpy_fallback(x: np.ndarray, sizes: np.ndarray, w: int) -> np.ndarray:
    ends = np.cumsum(sizes)
    starts = ends - sizes
    out = np.full((sizes.shape[0], x.shape[1]), -np.inf, dtype=np.float32)
    for i in range(sizes.shape[0]):
        c = int(sizes[i]) - w + 1
        if c > 0:
            out[i] = x[int(starts[i]) : int(starts[i]) + c].max(axis=0)
    return out


def kernel(x, sizes, window_size) -> np.ndarray:
    x = np.ascontiguousarray(np.asarray(x, dtype=np.float32))
    sizes = np.asarray(sizes)
    w = int(np.asarray(window_size))
    n_seg = sizes.shape[0]
    count = SEG_LEN - w + 1

    uniform = (
        x.ndim == 2
        and x.shape[1] == D
        and bool((sizes == SEG_LEN).all())
        and x.shape[0] == n_seg * SEG_LEN
        and n_seg % (N_CORES * SEGS_PER_TILE) == 0
        and (n_seg // N_CORES) % 128 == 0
        and SEG_LEN - J < count <= SEG_LEN
    )
    if not uniform:
        return _numpy_fallback(x, sizes, w)

    n_seg_core = n_seg // N_CORES
    key = (n_seg_core, count)
    if key not in _PROGRAM_CACHE:
        _PROGRAM_CACHE[key] = _build_program(n_seg_core, count)
    nc = _PROGRAM_CACHE[key]

    shards = np.split(x, N_CORES, axis=0)
    in_maps = [{"x": s} for s in shards]
    res = run_bass_kernel_spmd(nc, in_maps, core_ids=list(range(N_CORES)))
    return np.concatenate([r["out"] for r in res.results], axis=0)


# revision 11
# speedup vs baseline: 1.2036x; 1.2036x over previous
"""Segment-prefix max kernel for Trainium2 (8 NeuronCores, SPMD).

Problem: x [1048576, 128] f32, 2048 uniform segments of 512 rows each;
out[i, :] = max over the first (512 - window_size + 1) rows of segment i.

Strategy (memory-bound, ~512 MiB streamed from HBM):
  - Shard segments across 8 cores: core c gets rows [c*131072, (c+1)*131072)
    and produces out rows [c*256, (c+1)*256). No cross-core communication.
  - Per core, 4 MiB tiles of 16 segments: partition p holds runs p and
    128+p of the tile's 256 consecutive 32-row runs — 16 KiB contiguous
    DMA runs (vs 2 KiB naive), few large transfers, alternating between
    the sync and scalar HWDGE rings so all 16 DMA engines stay fed.
  - Run 15 of each segment includes the invalid window tail (rows >= count),
    so it is EXCLUDED from the per-tile reduce. The valid tail rows
    [count-32, count) of all segments are instead loaded once up front
    (2 x 2 MiB), folded separately, and joined into the output columns.
  - The 32 -> 1 fold along the free axis runs on DVE as a binary tree; the
    first level reads f32 and writes bf16, middle levels run in bf16 at 2x
    DVE throughput, the last level emits f32 (rel tolerance 2e-2 >> bf16's
    ~4e-3 rounding).
  - Cross-partition max (each segment = 16 consecutive partitions of one
    fill) goes through a PE transpose (identity matmul into PSUM) and one
    DVE reduce_max over each segment's first 15 columns, max'd with the
    precomputed tail column.
  - Columns accumulate in an SBUF [128, n_seg] f32 buffer, PE-transposed
    back to row-major [n_seg, 128] chunks and DMA'd out.
"""

import numpy as np

import concourse.bacc as bacc
import concourse.bass as bass
import concourse.tile as tile
from concourse import mybir
from concourse.bass_utils import run_bass_kernel_spmd
from concourse.masks import make_identity

N_CORES = 8
SEG_LEN = 512
D = 128
J = 32  # rows per run (16 KiB contiguous DMA run)
RUNS = SEG_LEN // J  # 16 runs (partitions) per segment
FILLS = 2  # 128-partition fills per tile; tile = FILLS * 2 MiB
SEGS_PER_FILL = 128 // RUNS  # 8
SEGS_PER_TILE = FILLS * SEGS_PER_FILL  # 16 segments, 4 MiB tiles

_PROGRAM_CACHE: dict = {}


def _build_program(n_seg_core: int, count: int) -> bacc.Bacc:
    """Bass program for one core: n_seg_core segments, max over first
    `count` rows of each. Requires SEG_LEN - J < count <= SEG_LEN."""
    assert SEG_LEN - J < count <= SEG_LEN
    rows = n_seg_core * SEG_LEN
    n_tiles = n_seg_core // SEGS_PER_TILE
    n_slot = n_seg_core // 128  # tail chunks of 128 segments
    has_tail = count < SEG_LEN
    f32 = mybir.dt.float32
    bf16 = mybir.dt.bfloat16

    nc = bacc.Bacc("TRN2", target_bir_lowering=False, debug=False)
    x_in = nc.dram_tensor("x", [rows, D], f32, kind="ExternalInput")
    out_t = nc.dram_tensor("out", [n_seg_core, D], f32, kind="ExternalOutput")

    # tile t, partition p, fill f -> run 256*t + 128*f + p
    x_tile = x_in.rearrange("(t f p j) d -> t p f j d", f=FILLS, p=128, j=J)
    # tail view: partition p, chunk a -> rows of segment a*128+p
    x_tail = x_in.rearrange("(a p q) d -> p a q d", p=128, q=SEG_LEN)

    rings = [nc.sync, nc.scalar]

    with tile.TileContext(nc) as tc:
        with (
            tc.tile_pool(name="io", bufs=4) as io_pool,
            tc.tile_pool(name="work", bufs=3) as work_pool,
            tc.tile_pool(name="tailp", bufs=1) as tail_pool,
            tc.tile_pool(name="psum", bufs=4, space="PSUM") as psum_pool,
            tc.tile_pool(name="psum2", bufs=2, space="PSUM") as psum_pool2,
            tc.tile_pool(name="consts", bufs=1) as consts,
        ):
            ident_f = consts.tile([128, 128], f32)
            make_identity(nc, ident_f)
            outbuf = consts.tile([128, n_seg_core], f32)

            if has_tail:
                # upfront: fold the valid tail rows of every segment
                taibuf = consts.tile([128, n_seg_core], f32)
                wtf = tail_pool.tile([128, n_slot, D], f32)
                for a in range(n_slot):
                    tt = tail_pool.tile([128, J, D], f32, tag="tt")
                    rings[a % 2].dma_start(
                        out=tt, in_=x_tail[:, a, count - J : count]
                    )
                    wt = tail_pool.tile([128, J // 2, D], bf16, tag="wt")
                    nc.vector.tensor_max(
                        out=wt, in0=tt[:, : J // 2], in1=tt[:, J // 2 :]
                    )
                    k = J // 2
                    while k > 2:
                        k //= 2
                        nc.vector.tensor_max(
                            out=wt[:, :k], in0=wt[:, :k], in1=wt[:, k : 2 * k]
                        )
                    nc.vector.tensor_max(
                        out=wtf[:, a], in0=wt[:, 0], in1=wt[:, 1]
                    )
                    ptt = psum_pool2.tile([128, 128], f32, tag="ptt")
                    nc.tensor.transpose(ptt, wtf[:, a], ident_f)
                    nc.scalar.copy(taibuf[:, a * 128 : (a + 1) * 128], ptt)

            for t in range(n_tiles):
                tl = io_pool.tile([128, FILLS, J, D], f32, tag="tl")
                g0 = t * SEGS_PER_TILE
                rings[t % 2].dma_start(out=tl, in_=x_tile[t])

                # fold 32 -> 1 along j: f32 -> bf16, bf16 tree, bf16 -> f32
                w = work_pool.tile([128, FILLS, J // 2, D], bf16, tag="w")
                nc.vector.tensor_max(
                    out=w, in0=tl[:, :, : J // 2], in1=tl[:, :, J // 2 :]
                )
                k = J // 2
                while k > 2:
                    k //= 2
                    nc.vector.tensor_max(
                        out=w[:, :, :k], in0=w[:, :, :k], in1=w[:, :, k : 2 * k]
                    )
                wf = work_pool.tile([128, FILLS, D], f32, tag="wf")
                nc.vector.tensor_max(out=wf, in0=w[:, :, 0], in1=w[:, :, 1])

                # per fill: transpose, reduce valid columns, join tail
                for f in range(FILLS):
                    gf = g0 + f * SEGS_PER_FILL
                    pt = psum_pool.tile([128, SEGS_PER_FILL, RUNS], f32, tag="pt")
                    nc.tensor.transpose(
                        pt.rearrange("p a b -> p (a b)"), wf[:, f], ident_f
                    )
                    if has_tail:
                        tmp = work_pool.tile([128, SEGS_PER_FILL], f32, tag="tmp")
                        nc.vector.reduce_max(
                            out=tmp,
                            in_=pt[:, :, : RUNS - 1],
                            axis=mybir.AxisListType.X,
                        )
                        nc.vector.tensor_max(
                            out=outbuf[:, gf : gf + SEGS_PER_FILL],
                            in0=tmp,
                            in1=taibuf[:, gf : gf + SEGS_PER_FILL],
                        )
                    else:
                        nc.vector.reduce_max(
                            out=outbuf[:, gf : gf + SEGS_PER_FILL],
                            in_=pt,
                            axis=mybir.AxisListType.X,
                        )

            # outbuf is [128 d, n_seg_core]; transpose back to [seg, d]
            for c in range(n_seg_core // 128):
                pt = psum_pool2.tile([128, 128], f32, tag="ot_ps")
                nc.tensor.transpose(pt, outbuf[:, c * 128 : (c + 1) * 128], ident_f)
                ot = io_pool.tile([128, 128], f32, tag="ot")
                nc.scalar.copy(ot, pt)
                nc.sync.dma_start(out=out_t[c * 128 : (c + 1) * 128, :], in_=ot)
    nc.compile()
    return nc


def _numpy_fallback(x: np.ndarray, sizes: np.ndarray, w: int) -> np.ndarray:
    ends = np.cumsum(sizes)
    starts = ends - sizes
    out = np.full((sizes.shape[0], x.shape[1]), -np.inf, dtype=np.float32)
    for i in range(sizes.shape[0]):
        c = int(sizes[i]) - w + 1
        if c > 0:
            out[i] = x[int(starts[i]) : int(starts[i]) + c].max(axis=0)
    return out


def kernel(x, sizes, window_size) -> np.ndarray:
    x = np.ascontiguousarray(np.asarray(x, dtype=np.float32))
    sizes = np.asarray(sizes)
    w = int(np.asarray(window_size))
    n_seg = sizes.shape[0]
    count = SEG_LEN - w + 1

    uniform = (
        x.ndim == 2
        and x.shape[1] == D
        and bool((sizes == SEG_LEN).all())
        and x.shape[0] == n_seg * SEG_LEN
        and n_seg % (N_CORES * SEGS_PER_TILE) == 0
        and (n_seg // N_CORES) % 128 == 0
        and SEG_LEN - J < count <= SEG_LEN
    )
    if not uniform:
        return _numpy_fallback(x, sizes, w)

    n_seg_core = n_seg // N_CORES
    key = (n_seg_core, count)
    if key not in _PROGRAM_CACHE:
        _PROGRAM_CACHE[key] = _build_program(n_seg_core, count)
    nc = _PROGRAM_CACHE[key]

    shards = np.split(x, N_CORES, axis=0)
    in_maps = [{"x": s} for s in shards]
    res = run_bass_kernel_spmd(nc, in_maps, core_ids=list(range(N_CORES)))
    return np.concatenate([r["out"] for r in res.results], axis=0)
